# revision 7
# baseline (speedup 1.0000x reference)
"""MLA forward kernel for Trainium2, 8 NeuronCores.

Sharding: data-parallel over batch (2) x tensor-parallel over heads (16 -> 4
groups of 4). Core c handles batch c//4, head group c%4. kv compression is
replicated per core. Each core emits a partial [S, D] output (its heads'
contribution through out_proj, already softmax-normalized); the host sums the
4 partials per batch.

Layouts on device (all matmul operands bf16, fp32 PSUM accumulation):
  xT    [D, S]   x transposed (host-prepped)
  kvT   [R, S]   latent, RMS-normed, transposed on-chip via PE transpose
  QnT/KnT [DN, S] per head; QrT/KrT [64, S] per head (RoPE applied pre-transpose)
  V     [S, 4*DV]
  scores^T tiles [k=128, q=512]; P^T = exp(scale*(s+mask)); AV gives outT [dv, q];
  row-sums via ones-vector matmul; 1/l broadcast via rank-1 matmul; out_proj
  contracts dv with wo rows.
"""

import sys
import numpy as np
import ml_dtypes

sys.path.insert(0, "/opt/trn_rl_repo")

import concourse.bass as bass  # noqa: E402
import concourse.tile as tile  # noqa: E402
from concourse import mybir, bacc  # noqa: E402
from concourse.bass_utils import run_bass_kernel_spmd  # noqa: E402
from concourse.masks import make_identity  # noqa: E402
from contextlib import ExitStack  # noqa: E402

B, S, D = 2, 2048, 2048
H, DN, DR, DV, R = 16, 128, 64, 128, 512
HL = 4  # heads per core
EPS = 1e-6
SCALE = 1.0 / float(np.sqrt(DN + DR))
BF = mybir.dt.bfloat16
F32 = mybir.dt.float32
NT = S // 128   # 16 s-tiles
NS = S // 512   # 4 s-supers
DCK = D // 128  # 16 D chunks
RCK = R // 128  # 4 R chunks

_CACHE = {}


def _rope(nc, tmp, dst, src_psum, cos_t, sin_t):
    # src [128, 256] (4 head-blocks x 64, interleaved pairs), cos/sin [128,128]
    e = src_psum[:, 0:256:2]
    o = src_psum[:, 1:256:2]
    de = dst[:, 0:256:2]
    do = dst[:, 1:256:2]
    t1 = tmp.tile([128, 128], F32, tag="t1", name="t1")
    t2 = tmp.tile([128, 128], F32, tag="t2", name="t2")
    nc.vector.tensor_mul(t1[:], e, cos_t)
    nc.vector.tensor_mul(t2[:], o, sin_t)
    nc.vector.tensor_sub(de, t1[:], t2[:])
    t3 = tmp.tile([128, 128], F32, tag="t3", name="t3")
    t4 = tmp.tile([128, 128], F32, tag="t4", name="t4")
    nc.vector.tensor_mul(t3[:], e, sin_t)
    nc.vector.tensor_mul(t4[:], o, cos_t)
    nc.vector.tensor_add(do, t3[:], t4[:])


def _build():
    nc = bacc.Bacc("TRN2", target_bir_lowering=False, debug=False)

    def din(name, shape, dt=BF):
        return nc.dram_tensor(name, list(shape), dt, kind="ExternalInput").ap()

    xT_d = din("xT", [D, S])
    wkv_d = din("wkv", [D, R])
    wqn_d = din("wqn", [D, HL * DN])
    wqr_d = din("wqr", [D, HL * DR])
    wkn_d = din("wkn", [R, HL * DN])
    wkr_d = din("wkr", [R, HL * DR])
    wv_d = din("wv", [R, HL * DV])
    wo_d = din("wo", [HL * DV, D])
    masks_d = din("masks", [4 * 128, 512], F32)
    cos_d = din("cos4", [S, 128], F32)
    sin_d = din("sin4", [S, 128], F32)
    out_d = nc.dram_tensor("out", [S, D], F32, kind="ExternalOutput").ap()

    with tile.TileContext(nc) as tc, ExitStack() as outer:
        # ---- persistent pools
        pp = outer.enter_context(tc.tile_pool(name="persist", bufs=1))
        ones_t = pp.tile([128, 1], BF, tag="ones", name="ones")
        ones1_t = pp.tile([1, 128], F32, tag="ones1", name="ones1")
        ident = pp.tile([128, 128], BF, tag="ident", name="ident")
        eps_t = pp.tile([128, 1], F32, tag="eps", name="eps")
        qk = outer.enter_context(tc.tile_pool(name="qk", bufs=1))
        QnT = [qk.tile([128, S], BF, tag=f"QnT{m}", name=f"QnT{m}") for m in range(HL)]
        QrT = [qk.tile([128, S], BF, tag=f"QrT{m}", name=f"QrT{m}") for m in range(2)]
        KnT = [qk.tile([128, S], BF, tag=f"KnT{m}", name=f"KnT{m}") for m in range(HL)]
        KrT = [qk.tile([128, S], BF, tag=f"KrT{m}", name=f"KrT{m}") for m in range(2)]
        Vt = [qk.tile([128, HL * DV], BF, tag=f"V{i}", name=f"V{i}") for i in range(NT)]

        nc.vector.memset(eps_t[:], EPS)
        nc.vector.memset(ones_t[:], 1.0)
        nc.vector.memset(ones1_t[:], 1.0)
        make_identity(nc, ident[:])

        # ================= Phase 1: x-side projections =================
        kv_scope = ExitStack()
        p_kvT = kv_scope.enter_context(tc.tile_pool(name="kvTp", bufs=1))
        kvT = [p_kvT.tile([128, S], BF, tag=f"kvT{r}", name=f"kvT{r}")
               for r in range(RCK)]
        with ExitStack() as ph1:
            p_xt = ph1.enter_context(tc.tile_pool(name="xt", bufs=1))
            p_w1 = ph1.enter_context(tc.tile_pool(name="w1", bufs=1))
            p_cs = ph1.enter_context(tc.tile_pool(name="cs1", bufs=3))
            p_sc = ph1.enter_context(tc.tile_pool(name="scr1", bufs=3))
            p_tmp = ph1.enter_context(tc.tile_pool(name="tmp1", bufs=3))
            p_ps = ph1.enter_context(tc.tile_pool(name="ps1", bufs=2, space="PSUM"))
            p_pt = ph1.enter_context(tc.tile_pool(name="pt1", bufs=2, space="PSUM"))

            xt = [p_xt.tile([128, S], BF, tag=f"x{k}", name=f"x{k}") for k in range(DCK)]
            wkv = [p_w1.tile([128, R], BF, tag=f"kv{k}", name=f"kv{k}") for k in range(DCK)]
            wqn = [p_w1.tile([128, HL * DN], BF, tag=f"qn{k}", name=f"qn{k}") for k in range(DCK)]
            wqr = [p_w1.tile([128, HL * DR], BF, tag=f"qr{k}", name=f"qr{k}") for k in range(DCK)]
            for k in range(DCK):
                sl = slice(128 * k, 128 * (k + 1))
                nc.sync.dma_start(xt[k][:], xT_d[sl, :])
                nc.sync.dma_start(wkv[k][:], wkv_d[sl, :])
                nc.sync.dma_start(wqn[k][:], wqn_d[sl, :])
                nc.sync.dma_start(wqr[k][:], wqr_d[sl, :])

            # kv = rmsnorm(x @ wkv), then transpose -> kvT
            for i in range(NT):
                ps = p_ps.tile([128, R], F32, tag="pkv", name="pkv")
                for k in range(DCK):
                    nc.tensor.matmul(ps[:], xt[k][:, 128 * i:128 * (i + 1)], wkv[k][:],
                                     start=(k == 0), stop=(k == DCK - 1))
                sq = p_sc.tile([128, R], F32, tag="sq", name="sq")
                var = p_sc.tile([128, 1], F32, tag="var", name="var")
                nc.scalar.activation(sq[:], ps[:], mybir.ActivationFunctionType.Square,
                                     accum_out=var[:])
                std = p_sc.tile([128, 1], F32, tag="std", name="std")
                nc.scalar.activation(std[:], var[:], mybir.ActivationFunctionType.Sqrt,
                                     scale=1.0 / R, bias=eps_t[:])
                rstd = p_sc.tile([128, 1], F32, tag="rstd", name="rstd")
                nc.vector.reciprocal(rstd[:], std[:])
                kvn = p_sc.tile([128, R], BF, tag="kvn", name="kvn")
                nc.vector.tensor_scalar_mul(kvn[:], ps[:], rstd[:])
                for r in range(RCK):
                    pt = p_pt.tile([128, 128], BF, tag="pt", name="pt")
                    nc.tensor.transpose(pt[:], kvn[:, 128 * r:128 * (r + 1)], ident[:])
                    nc.vector.tensor_copy(kvT[r][:, 128 * i:128 * (i + 1)], pt[:])

            # QnT[m] = (x @ wqn[:, m])^T
            for m in range(HL):
                for j in range(NS):
                    ps = p_ps.tile([128, 512], F32, tag="pq", name="pq")
                    for k in range(DCK):
                        nc.tensor.matmul(ps[:], wqn[k][:, 128 * m:128 * (m + 1)],
                                         xt[k][:, 512 * j:512 * (j + 1)],
                                         start=(k == 0), stop=(k == DCK - 1))
                    nc.vector.tensor_copy(QnT[m][:, 512 * j:512 * (j + 1)], ps[:])

            # q_rope: s-layout, rope, transpose
            for i in range(NT):
                ps = p_ps.tile([128, HL * DR], F32, tag="pqr", name="pqr")
                for k in range(DCK):
                    nc.tensor.matmul(ps[:], xt[k][:, 128 * i:128 * (i + 1)], wqr[k][:],
                                     start=(k == 0), stop=(k == DCK - 1))
                ct = p_cs.tile([128, 128], F32, tag="ct", name="ct")
                st = p_cs.tile([128, 128], F32, tag="st", name="st")
                nc.sync.dma_start(ct[:], cos_d[128 * i:128 * (i + 1), :])
                nc.sync.dma_start(st[:], sin_d[128 * i:128 * (i + 1), :])
                rp = p_sc.tile([128, HL * DR], BF, tag="rp", name="rp")
                _rope(nc, p_tmp, rp[:], ps[:], ct[:], st[:])
                for r2 in range(2):
                    pt = p_pt.tile([128, 128], BF, tag="pt", name="pt")
                    nc.tensor.transpose(pt[:], rp[:, 128 * r2:128 * (r2 + 1)], ident[:])
                    nc.vector.tensor_copy(QrT[r2][:, 128 * i:128 * (i + 1)], pt[:])

        # ================= Phase 2: latent up-projections =================
        with ExitStack() as ph2:
            p_w2 = ph2.enter_context(tc.tile_pool(name="w2", bufs=1))
            p_cs2 = ph2.enter_context(tc.tile_pool(name="cs2", bufs=3))
            p_sc2 = ph2.enter_context(tc.tile_pool(name="scr2", bufs=3))
            p_tmp2 = ph2.enter_context(tc.tile_pool(name="tmp2", bufs=3))
            p_ps2 = ph2.enter_context(tc.tile_pool(name="ps2", bufs=2, space="PSUM"))
            p_pt2 = ph2.enter_context(tc.tile_pool(name="pt2", bufs=2, space="PSUM"))

            wkn = [p_w2.tile([128, HL * DN], BF, tag=f"kn{r}", name=f"kn{r}") for r in range(RCK)]
            wkr = [p_w2.tile([128, HL * DR], BF, tag=f"kr{r}", name=f"kr{r}") for r in range(RCK)]
            wv = [p_w2.tile([128, HL * DV], BF, tag=f"v{r}", name=f"v{r}") for r in range(RCK)]
            for r in range(RCK):
                sl = slice(128 * r, 128 * (r + 1))
                nc.sync.dma_start(wkn[r][:], wkn_d[sl, :])
                nc.sync.dma_start(wkr[r][:], wkr_d[sl, :])
                nc.sync.dma_start(wv[r][:], wv_d[sl, :])

            for m in range(HL):  # KnT
                for j in range(NS):
                    ps = p_ps2.tile([128, 512], F32, tag="pk", name="pk")
                    for r in range(RCK):
                        nc.tensor.matmul(ps[:], wkn[r][:, 128 * m:128 * (m + 1)],
                                         kvT[r][:, 512 * j:512 * (j + 1)],
                                         start=(r == 0), stop=(r == RCK - 1))
                    nc.vector.tensor_copy(KnT[m][:, 512 * j:512 * (j + 1)], ps[:])

            for i in range(NT):  # k_rope + V
                ps = p_ps2.tile([128, HL * DR], F32, tag="pkr", name="pkr")
                for r in range(RCK):
                    nc.tensor.matmul(ps[:], kvT[r][:, 128 * i:128 * (i + 1)], wkr[r][:],
                                     start=(r == 0), stop=(r == RCK - 1))
                ct = p_cs2.tile([128, 128], F32, tag="ct", name="ct")
                st = p_cs2.tile([128, 128], F32, tag="st", name="st")
                nc.sync.dma_start(ct[:], cos_d[128 * i:128 * (i + 1), :])
                nc.sync.dma_start(st[:], sin_d[128 * i:128 * (i + 1), :])
                rp = p_sc2.tile([128, HL * DR], BF, tag="rp", name="rp")
                _rope(nc, p_tmp2, rp[:], ps[:], ct[:], st[:])
                for r2 in range(2):
                    pt = p_pt2.tile([128, 128], BF, tag="pt", name="pt")
                    nc.tensor.transpose(pt[:], rp[:, 128 * r2:128 * (r2 + 1)], ident[:])
                    nc.vector.tensor_copy(KrT[r2][:, 128 * i:128 * (i + 1)], pt[:])

                psv = p_ps2.tile([128, HL * DV], F32, tag="pv", name="pv")
                for r in range(RCK):
                    nc.tensor.matmul(psv[:], kvT[r][:, 128 * i:128 * (i + 1)], wv[r][:],
                                     start=(r == 0), stop=(r == RCK - 1))
                nc.vector.tensor_copy(Vt[i][:], psv[:])

        kv_scope.close()

        # ================= Phase 3: attention =================
        p_oT = outer.enter_context(tc.tile_pool(name="oT", bufs=1))
        outT = [p_oT.tile([128, 512], BF, tag=f"oT{i}", name=f"oT{i}")
                for i in range(HL * NS)]
        p_wo = outer.enter_context(tc.tile_pool(name="wop", bufs=1))
        wo_t = [p_wo.tile([128, D], BF, tag=f"wo{h}", name=f"wo{h}")
                for h in range(HL)]
        for h in range(HL):
            nc.sync.dma_start(wo_t[h][:], wo_d[128 * h:128 * (h + 1), :])
        with ExitStack() as ph3:
            p_mk = ph3.enter_context(tc.tile_pool(name="mk", bufs=1))
            mask_t = [p_mk.tile([128, 512], F32, tag=f"mk{p}", name=f"mk{p}")
                      for p in range(4)]
            for p in range(4):
                nc.sync.dma_start(mask_t[p][:], masks_d[128 * p:128 * (p + 1), :])
            p_pT = ph3.enter_context(tc.tile_pool(name="pT", bufs=4))
            p_sc3 = ph3.enter_context(tc.tile_pool(name="scr3", bufs=3))
            p_pss = ph3.enter_context(tc.tile_pool(name="pss", bufs=2, space="PSUM"))
            p_psav = ph3.enter_context(tc.tile_pool(name="psav", bufs=2, space="PSUM"))
            p_pssum = ph3.enter_context(tc.tile_pool(name="pssum", bufs=2, space="PSUM"))
            p_psb = ph3.enter_context(tc.tile_pool(name="psb", bufs=2, space="PSUM"))

            for h in range(HL):
                krt = KrT[h // 2]
                qrt = QrT[h // 2]
                ro = 64 * (h % 2)
                for j in range(NS):
                    ps_av = p_psav.tile([128, 512], F32, tag="av", name="av")
                    ps_sum = p_pssum.tile([1, 512], F32, tag="sum", name="sum")
                    nk = 4 * (j + 1)
                    for K in range(nk):
                        ps_s = p_pss.tile([128, 512], F32, tag="s", name="s")
                        nc.tensor.matmul(ps_s[:], KnT[h][:, 128 * K:128 * (K + 1)],
                                         QnT[h][:, 512 * j:512 * (j + 1)],
                                         start=True, stop=False)
                        nc.tensor.matmul(ps_s[:], krt[ro:ro + 64, 128 * K:128 * (K + 1)],
                                         qrt[ro:ro + 64, 512 * j:512 * (j + 1)],
                                         start=False, stop=True)
                        if K >= 4 * j:
                            nc.vector.tensor_add(ps_s[:], ps_s[:], mask_t[K - 4 * j][:])
                        pT = p_pT.tile([128, 512], BF, tag="pT", name="pT")
                        nc.scalar.activation(pT[:], ps_s[:],
                                             mybir.ActivationFunctionType.Exp,
                                             scale=SCALE)
                        nc.tensor.matmul(ps_av[:], Vt[K][:, 128 * h:128 * (h + 1)],
                                         pT[:], start=(K == 0), stop=(K == nk - 1))
                        nc.tensor.matmul(ps_sum[:], ones_t[:], pT[:],
                                         start=(K == 0), stop=(K == nk - 1))
                    rsum = p_sc3.tile([1, 512], F32, tag="rs", name="rs")
                    nc.vector.reciprocal(rsum[:], ps_sum[:])
                    ps_b = p_psb.tile([128, 512], F32, tag="b", name="b")
                    nc.tensor.matmul(ps_b[:], ones1_t[:], rsum[:], start=True, stop=True)
                    sb_b = p_sc3.tile([128, 512], F32, tag="sbb", name="sbb")
                    nc.vector.tensor_copy(sb_b[:], ps_b[:])
                    nc.vector.tensor_mul(outT[h * NS + j][:], ps_av[:], sb_b[:])

        # ================= Phase 4: output projection =================
        with ExitStack() as ph4:
            p_fo = ph4.enter_context(tc.tile_pool(name="fo", bufs=3))
            p_psf = ph4.enter_context(tc.tile_pool(name="psf", bufs=2, space="PSUM"))
            for i in range(NT):
                j, c = i // 4, 128 * (i % 4)
                for dsl in range(4):
                    ps = p_psf.tile([128, 512], F32, tag="f", name="f")
                    for h in range(HL):
                        nc.tensor.matmul(ps[:], outT[h * NS + j][:, c:c + 128],
                                         wo_t[h][:, 512 * dsl:512 * (dsl + 1)],
                                         start=(h == 0), stop=(h == HL - 1))
                    fo = p_fo.tile([128, 512], F32, tag="fo", name="fo")
                    nc.vector.tensor_copy(fo[:], ps[:])
                    nc.sync.dma_start(
                        out_d[128 * i:128 * (i + 1), 512 * dsl:512 * (dsl + 1)], fo[:])

    nc.compile()
    return nc


def _prep_inputs(x, freqs, w_kv, g_kv, w_k, w_v, w_qn, w_qr, w_o):
    bf = ml_dtypes.bfloat16
    f32 = np.float32
    wk3 = (w_k.astype(f32) * g_kv.astype(f32)[:, None]).reshape(R, H, DN + DR)
    wv2 = (w_v.astype(f32) * g_kv.astype(f32)[:, None]).reshape(R, H, DV)
    # rope tables: packed e-view col j (of 128) has angle freqs[s, j % 32]
    ang = freqs.astype(f32)  # [S, 32]
    cos4 = np.cos(ang)[:, np.tile(np.arange(32), 4)].astype(f32)
    sin4 = np.sin(ang)[:, np.tile(np.arange(32), 4)].astype(f32)
    cos4 = np.ascontiguousarray(cos4)
    sin4 = np.ascontiguousarray(sin4)
    kp = np.arange(128)[:, None]
    qf = np.arange(512)[None, :]
    masks = np.concatenate(
        [np.where(128 * p + kp <= qf, 0.0, -1e30).astype(f32) for p in range(4)], 0)
    in_maps = []
    for c in range(8):
        b, g = c // 4, c % 4
        hs = slice(4 * g, 4 * g + 4)
        m = {
            "xT": np.ascontiguousarray(x[b].astype(f32).T).astype(bf),
            "wkv": w_kv.astype(bf),
            "wqn": np.ascontiguousarray(
                w_qn.reshape(D, H, DN)[:, hs].reshape(D, HL * DN)).astype(bf),
            "wqr": np.ascontiguousarray(
                w_qr.reshape(D, H, DR)[:, hs].reshape(D, HL * DR)).astype(bf),
            "wkn": np.ascontiguousarray(
                wk3[:, hs, :DN].reshape(R, HL * DN)).astype(bf),
            "wkr": np.ascontiguousarray(
                wk3[:, hs, DN:].reshape(R, HL * DR)).astype(bf),
            "wv": np.ascontiguousarray(wv2[:, hs].reshape(R, HL * DV)).astype(bf),
            "wo": np.ascontiguousarray(
                w_o.reshape(H, DV, D)[hs].reshape(HL * DV, D)).astype(bf),
            "masks": masks,
            "cos4": cos4,
            "sin4": sin4,
        }
        in_maps.append(m)
    return in_maps


def kernel(x, freqs, w_kv, g_kv, w_k, w_v, w_qn, w_qr, w_o):
    if "nc" not in _CACHE:
        _CACHE["nc"] = _build()
    nc = _CACHE["nc"]
    in_maps = _prep_inputs(np.asarray(x), np.asarray(freqs), np.asarray(w_kv),
                           np.asarray(g_kv), np.asarray(w_k), np.asarray(w_v),
                           np.asarray(w_qn), np.asarray(w_qr), np.asarray(w_o))
    res = run_bass_kernel_spmd(nc, in_maps, list(range(8)), trace=False)
    out = np.zeros((B, S, D), np.float32)
    for c in range(8):
        out[c // 4] += res.results[c]["out"]
    return out


# revision 9
# speedup vs baseline: 1.1233x; 1.1233x over previous
"""MLA forward kernel for Trainium2, 8 NeuronCores.

Sharding: data-parallel over batch (2) x tensor-parallel over heads (16 -> 4
groups of 4). Core c handles batch c//4, head group c%4. kv compression is
replicated per core. Each core emits a partial [S, D] output (its heads'
contribution through out_proj, already softmax-normalized); the host sums the
4 partials per batch.

Layouts on device (all matmul operands bf16, fp32 PSUM accumulation):
  xT    [D, S]   x transposed (host-prepped)
  kvT   [R, S]   latent, RMS-normed, transposed on-chip via PE transpose
  QnT/KnT [DN, S] per head; QrT/KrT [64, S] per head (RoPE applied pre-transpose)
  V     [S, 4*DV]
  scores^T tiles [k=128, q=512]; P^T = exp(scale*(s+mask)); AV gives outT [dv, q];
  row-sums via ones-vector matmul; 1/l broadcast via rank-1 matmul; out_proj
  contracts dv with wo rows.
"""

import sys
import numpy as np
import ml_dtypes

sys.path.insert(0, "/opt/trn_rl_repo")

import concourse.bass as bass  # noqa: E402
import concourse.tile as tile  # noqa: E402
from concourse import mybir, bacc  # noqa: E402
from concourse.bass_utils import run_bass_kernel_spmd  # noqa: E402
from concourse.masks import make_identity  # noqa: E402
from contextlib import ExitStack  # noqa: E402

B, S, D = 2, 2048, 2048
H, DN, DR, DV, R = 16, 128, 64, 128, 512
HL = 4  # heads per core
EPS = 1e-6
SCALE = 1.0 / float(np.sqrt(DN + DR))
BF = mybir.dt.bfloat16
F32 = mybir.dt.float32
NT = S // 128   # 16 s-tiles
NS = S // 512   # 4 s-supers
DCK = D // 128  # 16 D chunks
RCK = R // 128  # 4 R chunks

_CACHE = {}


def _rope(nc, tmp, dst, src_psum, cos_t, sin_t):
    # src [128, 256] (4 head-blocks x 64, interleaved pairs), cos/sin [128,128]
    e = src_psum[:, 0:256:2]
    o = src_psum[:, 1:256:2]
    de = dst[:, 0:256:2]
    do = dst[:, 1:256:2]
    t1 = tmp.tile([128, 128], F32, tag="t1", name="t1")
    t2 = tmp.tile([128, 128], F32, tag="t2", name="t2")
    nc.vector.tensor_mul(t1[:], e, cos_t)
    nc.vector.tensor_mul(t2[:], o, sin_t)
    nc.vector.tensor_sub(de, t1[:], t2[:])
    t3 = tmp.tile([128, 128], F32, tag="t3", name="t3")
    t4 = tmp.tile([128, 128], F32, tag="t4", name="t4")
    nc.vector.tensor_mul(t3[:], e, sin_t)
    nc.vector.tensor_mul(t4[:], o, cos_t)
    nc.vector.tensor_add(do, t3[:], t4[:])


def _build():
    nc = bacc.Bacc("TRN2", target_bir_lowering=False, debug=False)

    def din(name, shape, dt=BF):
        return nc.dram_tensor(name, list(shape), dt, kind="ExternalInput").ap()

    xT_d = din("xT", [D, S])
    wkv_d = din("wkv", [D, R])
    wqn_d = din("wqn", [D, HL * DN])
    wqr_d = din("wqr", [D, HL * DR])
    wkn_d = din("wkn", [R, HL * DN])
    wkr_d = din("wkr", [R, HL * DR])
    wv_d = din("wv", [R, HL * DV])
    wo_d = din("wo", [HL * DV, D])
    masks_d = din("masks", [4 * 128, 512], F32)
    cos_d = din("cos4", [S, 128], F32)
    sin_d = din("sin4", [S, 128], F32)
    out_d = nc.dram_tensor("out", [S, D], F32, kind="ExternalOutput").ap()

    with tile.TileContext(nc) as tc, ExitStack() as outer:
        # ---- persistent pools
        pp = outer.enter_context(tc.tile_pool(name="persist", bufs=1))
        ones_t = pp.tile([128, 1], BF, tag="ones", name="ones")
        ones1_t = pp.tile([1, 128], F32, tag="ones1", name="ones1")
        ident = pp.tile([128, 128], BF, tag="ident", name="ident")
        eps_t = pp.tile([128, 1], F32, tag="eps", name="eps")
        qk = outer.enter_context(tc.tile_pool(name="qk", bufs=1))
        QnT = [qk.tile([128, S], BF, tag=f"QnT{m}", name=f"QnT{m}") for m in range(HL)]
        QrT = [qk.tile([128, S], BF, tag=f"QrT{m}", name=f"QrT{m}") for m in range(2)]
        KnT = [qk.tile([128, S], BF, tag=f"KnT{m}", name=f"KnT{m}") for m in range(HL)]
        KrT = [qk.tile([128, S], BF, tag=f"KrT{m}", name=f"KrT{m}") for m in range(2)]
        Vt = [qk.tile([128, HL * DV], BF, tag=f"V{i}", name=f"V{i}") for i in range(NT)]

        nc.vector.memset(eps_t[:], EPS)
        nc.vector.memset(ones_t[:], 1.0)
        nc.vector.memset(ones1_t[:], 1.0)
        make_identity(nc, ident[:])

        # ================= Phase 1: x-side projections =================
        kv_scope = ExitStack()
        p_kvT = kv_scope.enter_context(tc.tile_pool(name="kvTp", bufs=1))
        kvT = [p_kvT.tile([128, S], BF, tag=f"kvT{r}", name=f"kvT{r}")
               for r in range(RCK)]
        with ExitStack() as ph1:
            p_xt = ph1.enter_context(tc.tile_pool(name="xt", bufs=1))
            p_w1 = ph1.enter_context(tc.tile_pool(name="w1", bufs=1))
            p_cs = ph1.enter_context(tc.tile_pool(name="cs1", bufs=3))
            p_sc = ph1.enter_context(tc.tile_pool(name="scr1", bufs=3))
            p_tmp = ph1.enter_context(tc.tile_pool(name="tmp1", bufs=3))
            p_ps = ph1.enter_context(tc.tile_pool(name="ps1", bufs=2, space="PSUM"))
            p_pt = ph1.enter_context(tc.tile_pool(name="pt1", bufs=2, space="PSUM"))

            xt = [p_xt.tile([128, S], BF, tag=f"x{k}", name=f"x{k}") for k in range(DCK)]
            wkv = [p_w1.tile([128, R], BF, tag=f"kv{k}", name=f"kv{k}") for k in range(DCK)]
            wqn = [p_w1.tile([128, HL * DN], BF, tag=f"qn{k}", name=f"qn{k}") for k in range(DCK)]
            wqr = [p_w1.tile([128, HL * DR], BF, tag=f"qr{k}", name=f"qr{k}") for k in range(DCK)]
            for k in range(DCK):
                sl = slice(128 * k, 128 * (k + 1))
                nc.sync.dma_start(xt[k][:], xT_d[sl, :])
                nc.sync.dma_start(wkv[k][:], wkv_d[sl, :])
                nc.sync.dma_start(wqn[k][:], wqn_d[sl, :])
                nc.sync.dma_start(wqr[k][:], wqr_d[sl, :])

            # kv = rmsnorm(x @ wkv), then transpose -> kvT
            for i in range(NT):
                ps = p_ps.tile([128, R], F32, tag="pkv", name="pkv")
                for k in range(DCK):
                    nc.tensor.matmul(ps[:], xt[k][:, 128 * i:128 * (i + 1)], wkv[k][:],
                                     start=(k == 0), stop=(k == DCK - 1))
                sq = p_sc.tile([128, R], F32, tag="sq", name="sq")
                var = p_sc.tile([128, 1], F32, tag="var", name="var")
                nc.scalar.activation(sq[:], ps[:], mybir.ActivationFunctionType.Square,
                                     accum_out=var[:])
                std = p_sc.tile([128, 1], F32, tag="std", name="std")
                nc.scalar.activation(std[:], var[:], mybir.ActivationFunctionType.Sqrt,
                                     scale=1.0 / R, bias=eps_t[:])
                rstd = p_sc.tile([128, 1], F32, tag="rstd", name="rstd")
                nc.vector.reciprocal(rstd[:], std[:])
                kvn = p_sc.tile([128, R], BF, tag="kvn", name="kvn")
                nc.vector.tensor_scalar_mul(kvn[:], ps[:], rstd[:])
                for r in range(RCK):
                    pt = p_pt.tile([128, 128], BF, tag="pt", name="pt")
                    nc.tensor.transpose(pt[:], kvn[:, 128 * r:128 * (r + 1)], ident[:])
                    nc.vector.tensor_copy(kvT[r][:, 128 * i:128 * (i + 1)], pt[:])

            # QnT[m] = (x @ wqn[:, m])^T
            for m in range(HL):
                for j in range(NS):
                    ps = p_ps.tile([128, 512], F32, tag="pq", name="pq")
                    for k in range(DCK):
                        nc.tensor.matmul(ps[:], wqn[k][:, 128 * m:128 * (m + 1)],
                                         xt[k][:, 512 * j:512 * (j + 1)],
                                         start=(k == 0), stop=(k == DCK - 1))
                    nc.vector.tensor_copy(QnT[m][:, 512 * j:512 * (j + 1)], ps[:])

            # q_rope: s-layout, rope, transpose
            for i in range(NT):
                ps = p_ps.tile([128, HL * DR], F32, tag="pqr", name="pqr")
                for k in range(DCK):
                    nc.tensor.matmul(ps[:], xt[k][:, 128 * i:128 * (i + 1)], wqr[k][:],
                                     start=(k == 0), stop=(k == DCK - 1))
                ct = p_cs.tile([128, 128], F32, tag="ct", name="ct")
                st = p_cs.tile([128, 128], F32, tag="st", name="st")
                nc.sync.dma_start(ct[:], cos_d[128 * i:128 * (i + 1), :])
                nc.sync.dma_start(st[:], sin_d[128 * i:128 * (i + 1), :])
                rp = p_sc.tile([128, HL * DR], BF, tag="rp", name="rp")
                _rope(nc, p_tmp, rp[:], ps[:], ct[:], st[:])
                for r2 in range(2):
                    pt = p_pt.tile([128, 128], BF, tag="pt", name="pt")
                    nc.tensor.transpose(pt[:], rp[:, 128 * r2:128 * (r2 + 1)], ident[:])
                    nc.vector.tensor_copy(QrT[r2][:, 128 * i:128 * (i + 1)], pt[:])

        # ================= Phase 2: latent up-projections =================
        with ExitStack() as ph2:
            p_w2 = ph2.enter_context(tc.tile_pool(name="w2", bufs=1))
            p_cs2 = ph2.enter_context(tc.tile_pool(name="cs2", bufs=3))
            p_sc2 = ph2.enter_context(tc.tile_pool(name="scr2", bufs=3))
            p_tmp2 = ph2.enter_context(tc.tile_pool(name="tmp2", bufs=3))
            p_ps2 = ph2.enter_context(tc.tile_pool(name="ps2", bufs=2, space="PSUM"))
            p_pt2 = ph2.enter_context(tc.tile_pool(name="pt2", bufs=2, space="PSUM"))

            wkn = [p_w2.tile([128, HL * DN], BF, tag=f"kn{r}", name=f"kn{r}") for r in range(RCK)]
            wkr = [p_w2.tile([128, HL * DR], BF, tag=f"kr{r}", name=f"kr{r}") for r in range(RCK)]
            wv = [p_w2.tile([128, HL * DV], BF, tag=f"v{r}", name=f"v{r}") for r in range(RCK)]
            for r in range(RCK):
                sl = slice(128 * r, 128 * (r + 1))
                nc.sync.dma_start(wkn[r][:], wkn_d[sl, :])
                nc.sync.dma_start(wkr[r][:], wkr_d[sl, :])
                nc.sync.dma_start(wv[r][:], wv_d[sl, :])

            for m in range(HL):  # KnT
                for j in range(NS):
                    ps = p_ps2.tile([128, 512], F32, tag="pk", name="pk")
                    for r in range(RCK):
                        nc.tensor.matmul(ps[:], wkn[r][:, 128 * m:128 * (m + 1)],
                                         kvT[r][:, 512 * j:512 * (j + 1)],
                                         start=(r == 0), stop=(r == RCK - 1))
                    nc.vector.tensor_copy(KnT[m][:, 512 * j:512 * (j + 1)], ps[:])

            for i in range(NT):  # k_rope + V
                ps = p_ps2.tile([128, HL * DR], F32, tag="pkr", name="pkr")
                for r in range(RCK):
                    nc.tensor.matmul(ps[:], kvT[r][:, 128 * i:128 * (i + 1)], wkr[r][:],
                                     start=(r == 0), stop=(r == RCK - 1))
                ct = p_cs2.tile([128, 128], F32, tag="ct", name="ct")
                st = p_cs2.tile([128, 128], F32, tag="st", name="st")
                nc.sync.dma_start(ct[:], cos_d[128 * i:128 * (i + 1), :])
                nc.sync.dma_start(st[:], sin_d[128 * i:128 * (i + 1), :])
                rp = p_sc2.tile([128, HL * DR], BF, tag="rp", name="rp")
                _rope(nc, p_tmp2, rp[:], ps[:], ct[:], st[:])
                for r2 in range(2):
                    pt = p_pt2.tile([128, 128], BF, tag="pt", name="pt")
                    nc.tensor.transpose(pt[:], rp[:, 128 * r2:128 * (r2 + 1)], ident[:])
                    nc.vector.tensor_copy(KrT[r2][:, 128 * i:128 * (i + 1)], pt[:])

                psv = p_ps2.tile([128, HL * DV], F32, tag="pv", name="pv")
                for r in range(RCK):
                    nc.tensor.matmul(psv[:], kvT[r][:, 128 * i:128 * (i + 1)], wv[r][:],
                                     start=(r == 0), stop=(r == RCK - 1))
                nc.vector.tensor_copy(Vt[i][:], psv[:])

        kv_scope.close()

        # ================= Phase 3: attention =================
        p_oT = outer.enter_context(tc.tile_pool(name="oT", bufs=1))
        outT = [p_oT.tile([128, 512], BF, tag=f"oT{i}", name=f"oT{i}")
                for i in range(HL * NS)]
        p_wo = outer.enter_context(tc.tile_pool(name="wop", bufs=1))
        wo_t = [p_wo.tile([128, D], BF, tag=f"wo{h}", name=f"wo{h}")
                for h in range(HL)]
        for h in range(HL):
            nc.sync.dma_start(wo_t[h][:], wo_d[128 * h:128 * (h + 1), :])
        with ExitStack() as ph3:
            p_mk = ph3.enter_context(tc.tile_pool(name="mk", bufs=1))
            mask_t = [p_mk.tile([128, 512], F32, tag=f"mk{p}", name=f"mk{p}")
                      for p in range(4)]
            for p in range(4):
                nc.sync.dma_start(mask_t[p][:], masks_d[128 * p:128 * (p + 1), :])
            p_pT = ph3.enter_context(tc.tile_pool(name="pT", bufs=6))
            p_fo = ph3.enter_context(tc.tile_pool(name="fo", bufs=3))
            p_psf = ph3.enter_context(tc.tile_pool(name="psf", bufs=1, space="PSUM"))
            p_sc3 = ph3.enter_context(tc.tile_pool(name="scr3", bufs=3))
            p_pss = ph3.enter_context(tc.tile_pool(name="pss", bufs=3, space="PSUM"))
            p_psav = ph3.enter_context(tc.tile_pool(name="psav", bufs=2, space="PSUM"))
            p_pssum = ph3.enter_context(tc.tile_pool(name="pssum", bufs=1, space="PSUM"))
            p_psb = ph3.enter_context(tc.tile_pool(name="psb", bufs=1, space="PSUM"))

            for j in range(NS):
                for h in range(HL):
                    krt = KrT[h // 2]
                    qrt = QrT[h // 2]
                    ro = 64 * (h % 2)
                    ps_av = p_psav.tile([128, 512], F32, tag="av", name="av")
                    ps_sum = p_pssum.tile([1, 512], F32, tag="sum", name="sum")
                    nk = 4 * (j + 1)
                    for K in range(nk):
                        ps_s = p_pss.tile([128, 512], F32, tag="s", name="s")
                        nc.tensor.matmul(ps_s[:], KnT[h][:, 128 * K:128 * (K + 1)],
                                         QnT[h][:, 512 * j:512 * (j + 1)],
                                         start=True, stop=False)
                        nc.tensor.matmul(ps_s[:], krt[ro:ro + 64, 128 * K:128 * (K + 1)],
                                         qrt[ro:ro + 64, 512 * j:512 * (j + 1)],
                                         start=False, stop=True)
                        if K >= 4 * j:
                            p = K - 4 * j
                            w = 128 * (p + 1)
                            nc.vector.tensor_add(ps_s[:, :w], ps_s[:, :w],
                                                 mask_t[p][:, :w])
                        pT = p_pT.tile([128, 512], BF, tag="pT", name="pT")
                        nc.scalar.activation(pT[:], ps_s[:],
                                             mybir.ActivationFunctionType.Exp,
                                             scale=SCALE)
                        nc.tensor.matmul(ps_av[:], Vt[K][:, 128 * h:128 * (h + 1)],
                                         pT[:], start=(K == 0), stop=(K == nk - 1))
                        nc.tensor.matmul(ps_sum[:], ones_t[:], pT[:],
                                         start=(K == 0), stop=(K == nk - 1))
                    rsum = p_sc3.tile([1, 512], F32, tag="rs", name="rs")
                    nc.vector.reciprocal(rsum[:], ps_sum[:])
                    ps_b = p_psb.tile([128, 512], F32, tag="b", name="b")
                    nc.tensor.matmul(ps_b[:], ones1_t[:], rsum[:], start=True, stop=True)
                    sb_b = p_sc3.tile([128, 512], F32, tag="sbb", name="sbb")
                    nc.vector.tensor_copy(sb_b[:], ps_b[:])
                    nc.vector.tensor_mul(outT[h * NS + j][:], ps_av[:], sb_b[:])

                # out-projection for this q-super (s-tiles 4j..4j+3)
                for ii in range(4):
                    i = 4 * j + ii
                    c = 128 * ii
                    for dsl in range(4):
                        ps = p_psf.tile([128, 512], F32, tag="f", name="f")
                        for h in range(HL):
                            nc.tensor.matmul(ps[:], outT[h * NS + j][:, c:c + 128],
                                             wo_t[h][:, 512 * dsl:512 * (dsl + 1)],
                                             start=(h == 0), stop=(h == HL - 1))
                        fo = p_fo.tile([128, 512], F32, tag="fo", name="fo")
                        nc.vector.tensor_copy(fo[:], ps[:])
                        nc.sync.dma_start(
                            out_d[128 * i:128 * (i + 1),
                                  512 * dsl:512 * (dsl + 1)], fo[:])

    nc.compile()
    return nc


def _prep_inputs(x, freqs, w_kv, g_kv, w_k, w_v, w_qn, w_qr, w_o):
    bf = ml_dtypes.bfloat16
    f32 = np.float32
    wk3 = (w_k.astype(f32) * g_kv.astype(f32)[:, None]).reshape(R, H, DN + DR)
    wv2 = (w_v.astype(f32) * g_kv.astype(f32)[:, None]).reshape(R, H, DV)
    # rope tables: packed e-view col j (of 128) has angle freqs[s, j % 32]
    ang = freqs.astype(f32)  # [S, 32]
    cos4 = np.cos(ang)[:, np.tile(np.arange(32), 4)].astype(f32)
    sin4 = np.sin(ang)[:, np.tile(np.arange(32), 4)].astype(f32)
    cos4 = np.ascontiguousarray(cos4)
    sin4 = np.ascontiguousarray(sin4)
    kp = np.arange(128)[:, None]
    qf = np.arange(512)[None, :]
    masks = np.concatenate(
        [np.where(128 * p + kp <= qf, 0.0, -1e30).astype(f32) for p in range(4)], 0)
    in_maps = []
    for c in range(8):
        b, g = c // 4, c % 4
        hs = slice(4 * g, 4 * g + 4)
        m = {
            "xT": np.ascontiguousarray(x[b].astype(f32).T).astype(bf),
            "wkv": w_kv.astype(bf),
            "wqn": np.ascontiguousarray(
                w_qn.reshape(D, H, DN)[:, hs].reshape(D, HL * DN)).astype(bf),
            "wqr": np.ascontiguousarray(
                w_qr.reshape(D, H, DR)[:, hs].reshape(D, HL * DR)).astype(bf),
            "wkn": np.ascontiguousarray(
                wk3[:, hs, :DN].reshape(R, HL * DN)).astype(bf),
            "wkr": np.ascontiguousarray(
                wk3[:, hs, DN:].reshape(R, HL * DR)).astype(bf),
            "wv": np.ascontiguousarray(wv2[:, hs].reshape(R, HL * DV)).astype(bf),
            "wo": np.ascontiguousarray(
                w_o.reshape(H, DV, D)[hs].reshape(HL * DV, D)).astype(bf),
            "masks": masks,
            "cos4": cos4,
            "sin4": sin4,
        }
        in_maps.append(m)
    return in_maps


def kernel(x, freqs, w_kv, g_kv, w_k, w_v, w_qn, w_qr, w_o):
    if "nc" not in _CACHE:
        _CACHE["nc"] = _build()
    nc = _CACHE["nc"]
    in_maps = _prep_inputs(np.asarray(x), np.asarray(freqs), np.asarray(w_kv),
                           np.asarray(g_kv), np.asarray(w_k), np.asarray(w_v),
                           np.asarray(w_qn), np.asarray(w_qr), np.asarray(w_o))
    res = run_bass_kernel_spmd(nc, in_maps, list(range(8)), trace=False)
    out = np.zeros((B, S, D), np.float32)
    for c in range(8):
        out[c // 4] += res.results[c]["out"]
    return out
